# revision 1
# baseline (speedup 1.0000x reference)
"""Trainium2 Bass kernel for nn_GTLayer (sparse_attention problem).

Key structural fact about the reference: H == 1 and the softmax is taken
over the HEAD axis, so softmax(attn, axis=0) on a (1, N, N) tensor is
identically 1.0.  Therefore attn @ v reduces to broadcasting the column
sums of v to every row: the A mask, q and k projections are all dead
code.  The attention output row is a single constant vector

    base = (sum_i h_i) @ vw + N * vb, then @ ow + ob

which we compute exactly on the host.  Folding both BatchNorms (eval
mode -> per-feature affine) and the residuals, the whole layer is

    y = h2 + relu(h2 @ W1 + b1) @ W2 + C        (per-feature constants)

with h2 = h * sP.  The large constant part of t = relu(h2 @ W1 + b1) is
tc = relu(b1) (h2 is zero-mean): the device computes tv = t - tc in bf16
(small values -> accurate) and the exact tc @ W2 + C contribution rides
in the f32 h2C tensor, added on the vector engine.

Device pipeline per core (1024 rows):
  mm1:  zT = W1^T @ h2T            (PE, bf16, psum f32)
  ACT:  u  = relu(z + b1)          (per-partition bias, psum -> sbuf f32)
  DVE:  tv = u - tc  -> bf16
  mm2:  F  = tv @ W2               (PE, bf16, psum f32)
  DVE:  y  = F + h2C               (psum + sbuf f32)
  DMA out.

Rows (N=8192) are sharded over the 8 cores; weights are replicated.
DMA emission order puts row-group-0 activations and W1/W2 first so the
PE can start ~6us in; a chain of tiny warm-up matmuls keeps the PE HAM
unthrottled during the load phase.
"""

import numpy as np
from contextlib import ExitStack

import ml_dtypes
import concourse.bass as bass
import concourse.mybir as mybir
import concourse.tile as tile
from concourse import bacc
from concourse.bass_utils import run_bass_kernel_spmd

N = 8192
D = 512
H1 = 1024
NCORES = 8
RPC = N // NCORES  # rows per core
EPS = 1e-5
N_WARMUP = 7

BF16 = mybir.dt.bfloat16
F32 = mybir.dt.float32
NPBF16 = np.dtype(ml_dtypes.bfloat16)


def build_bass():
    nc = bacc.Bacc(
        "TRN2", target_bir_lowering=False, debug=False, num_devices=NCORES
    )
    h2T = nc.dram_tensor("h2t", [D, RPC], BF16, kind="ExternalInput")
    h2C = nc.dram_tensor("h2c", [RPC, D], F32, kind="ExternalInput")
    W1 = nc.dram_tensor("w1", [D, H1], BF16, kind="ExternalInput")
    W2 = nc.dram_tensor("w2", [H1, D], BF16, kind="ExternalInput")
    # b1 (cols 0..7) and tc (cols 8..15) packed: one DMA trigger
    BC = nc.dram_tensor("bc", [128, 2 * (H1 // 128)], F32, kind="ExternalInput")
    Y = nc.dram_tensor("y", [RPC, D], F32, kind="ExternalOutput")

    NC1 = H1 // 128  # 8 n-chunks in mm1 / k-chunks in mm2
    KC1 = D // 128   # 4 k-chunks in mm1
    RT = RPC // 128  # 8 row tiles
    RG = RPC // 512  # 2 row groups (mm1 free dim 512)

    with ExitStack() as ctx:
        tc = ctx.enter_context(tile.TileContext(nc))
        consts = ctx.enter_context(tc.tile_pool(name="consts", bufs=1))
        acts = ctx.enter_context(tc.tile_pool(name="acts", bufs=1))
        zpsum = ctx.enter_context(tc.tile_pool(name="zpsum", bufs=2, space="PSUM"))
        fpsum = ctx.enter_context(tc.tile_pool(name="fpsum", bufs=4, space="PSUM"))
        wpsum = ctx.enter_context(tc.tile_pool(name="wpsum", bufs=1, space="PSUM"))
        upool = ctx.enter_context(tc.tile_pool(name="upool", bufs=3))
        ypool = ctx.enter_context(tc.tile_pool(name="ypool", bufs=3))

        # --- PE warm-up on a memset tile: no DMA dependency, so the PE's
        # HAM activity window fills right after the preamble and real
        # matmuls run at 2.4 GHz instead of 1.2.
        wa = consts.tile([128, 512], BF16)
        nc.vector.memset(wa[:], 0.0)
        wp = wpsum.tile([128, 512], F32)
        for _ in range(N_WARMUP):
            nc.tensor.matmul(wp[:], wa[:, :128], wa[:], start=True, stop=True)

        # --- streaming inputs, critical-path order, few triggers ----------
        # each dma_start costs ~650ns serial trigger time on its engine's
        # queue; spread non-critical ones across otherwise-idle queues.
        bcsb = consts.tile([128, 2 * NC1], F32)
        nc.sync.dma_start(bcsb[:], BC[:, :])
        b1sb = bcsb[:, 0:NC1]
        tcsb = bcsb[:, NC1 : 2 * NC1]

        H2Tr = h2T.rearrange("(kc p) r -> p kc r", p=128)
        h2tsb = acts.tile([128, KC1, RPC], BF16)
        for kc in range(KC1):  # row-group 0 first: halves the critical load
            nc.sync.dma_start(h2tsb[:, kc, 0:512], H2Tr[:, kc, 0:512])
        w1sb = consts.tile([128, KC1, H1], BF16)
        W1r = W1.rearrange("(kc p) n -> p kc n", p=128)
        for nci in range(NC1):
            nc.sync.dma_start(
                w1sb[:, :, nci * 128 : (nci + 1) * 128],
                W1r[:, :, nci * 128 : (nci + 1) * 128],
            )
        for kc in range(KC1):  # row-group 1 activations
            nc.sync.dma_start(h2tsb[:, kc, 512:RPC], H2Tr[:, kc, 512:RPC])
        # W2 / h2C are needed later: keeping their triggers BEHIND the
        # critical h2T/W1 triggers on the same sync queue throttles them
        # (~650ns serial trigger each), so the critical transfers get the
        # HBM bandwidth first.  (Issuing them in parallel from the idle
        # gpsimd/scalar queues was measurably worse.)
        w2sb = consts.tile([128, NC1, D], BF16)
        W2r = W2.rearrange("(kc p) n -> p kc n", p=128)
        for nci in range(NC1):
            nc.sync.dma_start(w2sb[:, nci, :], W2r[:, nci, :])
        h2csb = acts.tile([128, RT, D], F32)
        H2Cr = h2C.rearrange("(rt p) f -> p rt f", p=128)
        for rt in range(RT):
            nc.sync.dma_start(h2csb[:, rt, :], H2Cr[:, rt, :])
        Yr = Y.rearrange("(rt p) f -> rt p f", p=128)

        # tv stored transposed: [n-in-chunk, n-chunk, row], bf16
        tvsb = acts.tile([128, NC1, RPC], BF16)

        for rg in range(RG):
            rs = rg * 512
            for nci in range(NC1):
                zp = zpsum.tile([128, 512], F32, tag="zp")
                for kc in range(KC1):
                    nc.tensor.matmul(
                        zp[:],
                        w1sb[:, kc, nci * 128 : (nci + 1) * 128],
                        h2tsb[:, kc, rs : rs + 512],
                        start=(kc == 0),
                        stop=(kc == KC1 - 1),
                    )
                u = upool.tile([128, 512], F32, tag="u")
                nc.scalar.activation(
                    u[:],
                    zp[:],
                    mybir.ActivationFunctionType.Relu,
                    bias=b1sb[:, nci : nci + 1],
                    scale=1.0,
                )
                nc.vector.tensor_scalar(
                    tvsb[:, nci, rs : rs + 512],
                    u[:],
                    tcsb[:, nci : nci + 1],
                    None,
                    mybir.AluOpType.subtract,
                )
            for rt in range(rg * (RT // RG), (rg + 1) * (RT // RG)):
                fp = fpsum.tile([128, D], F32, tag="fp")
                for nci in range(NC1):
                    nc.tensor.matmul(
                        fp[:],
                        tvsb[:, nci, rt * 128 : (rt + 1) * 128],
                        w2sb[:, nci, :],
                        start=(nci == 0),
                        stop=(nci == NC1 - 1),
                    )
                ysb = ypool.tile([128, D], F32, tag="ysb")
                nc.vector.tensor_tensor(
                    ysb[:], fp[:], h2csb[:, rt, :], mybir.AluOpType.add
                )
                nc.sync.dma_start(Yr[rt], ysb[:])
    nc.compile()
    return nc


_CACHE = {}


def _get_bass():
    if "nc" not in _CACHE:
        _CACHE["nc"] = build_bass()
    return _CACHE["nc"]


def _host_fold(inputs):
    """Fold attention shortcut + BNs into W1, b1, W2, h2, h2C (float64)."""
    f = lambda k: inputs[k].astype(np.float64)
    h = f("h")
    a1 = f("bn1_g") / np.sqrt(f("bn1_v") + EPS)
    c1 = f("bn1_b") - f("bn1_m") * a1
    a2 = f("bn2_g") / np.sqrt(f("bn2_v") + EPS)
    c2 = f("bn2_b") - f("bn2_m") * a2

    hs = h.sum(axis=0)
    s = hs @ f("vw") + N * f("vb")          # column sums of v
    base = s @ f("ow") + f("ob")            # constant attention-out row
    d1 = base * a1 + c1                     # constant row of bn1(x)
    sP = a1 * a2

    W1 = (1.0 / a2)[:, None] * f("f1w")
    b1 = (d1 @ f("f1w") + f("f1b")).astype(np.float32)
    W2 = f("f2w") * a2[None, :]
    C = (d1 + f("f2b")) * a2 + c2

    # device computes tv = relu(z + b1_f32) - tc_f32 in f32, so use the
    # exact same f32 constants when folding tc @ W2 into h2C
    tc = np.maximum(b1, 0.0)
    Cfull = C + tc.astype(np.float64) @ W2

    h2 = h * sP[None, :]
    pack = lambda v: v.reshape(H1 // 128, 128).T
    return {
        "W1": W1.astype(NPBF16),
        "bc": np.ascontiguousarray(np.concatenate([pack(b1), pack(tc)], axis=1)),
        "W2": W2.astype(NPBF16),
        "h2": h2.astype(np.float32),
        "h2C": (h2 + Cfull[None, :]).astype(np.float32),
    }


def make_in_maps(inputs):
    hf = _host_fold(inputs)
    h2bf = hf["h2"].astype(NPBF16)
    in_maps = []
    for c in range(NCORES):
        r0 = c * RPC
        in_maps.append(
            {
                "h2t": np.ascontiguousarray(h2bf[r0 : r0 + RPC].T),
                "h2c": hf["h2C"][r0 : r0 + RPC],
                "w1": hf["W1"],
                "w2": hf["W2"],
                "bc": hf["bc"],
            }
        )
    return in_maps


def kernel(**inputs):
    nc = _get_bass()
    in_maps = make_in_maps(inputs)
    res = run_bass_kernel_spmd(nc, in_maps, core_ids=list(range(NCORES)))
    return np.concatenate([r["y"] for r in res.results], axis=0)



# revision 4
# speedup vs baseline: 1.8021x; 1.8021x over previous
"""Trainium2 Bass kernel for nn_GTLayer (sparse_attention problem).

Structural facts exploited (all validated against the reference):

1. H == 1 and the softmax is over the HEAD axis, so softmax(attn, axis=0)
   on a (1, N, N) tensor is identically 1.0: the A mask and the q/k
   projections are dead code, and attention output is one constant row
   (column sums of v) computed exactly on the host.  Folding both eval-
   mode BatchNorms and residuals, the layer is

       y = h2 + relu(h2 @ W1 + b1) @ W2 + Cfull,   h2 = h * (a1*a2)

2. b1 = d1 @ f1w + f1b is dominated by the huge constant attention row
   (|b1| ~ 100) while the data term z = h2 @ W1 has |z| <= 3.75.  With
   the rigorous per-column Cauchy-Schwarz bound tau_j = max_r ||h2_r||
   * ||W1[:,j]||, columns split into:
     - always-on  (b1 >= tau, ~429): relu is identity -> folded on host
       into Wbig = I + W1_on @ W2_on (512x512, exact f64)
     - always-off (b1 <= -tau, ~423): relu(z+b1) == relu(b1) == const,
       tv == 0 -> dropped entirely
     - nonlinear  (~172, padded to 256): computed on device
   This halves the FLOPs and removes most of the mm1/relu work.

3. The output norm is dominated by the constant Cfull (rms ~143 vs data
   ~1.1), so fp8(e4m3) operands + f32 PSUM accumulate give ~1.7e-3
   relative error (measured on the exact inputs) vs the 2e-2 gate.
   fp8 DoubleRow matmuls run 2 contraction subtiles per instruction.

Device pipeline per core (1024 rows, everything transposed [feat, row]
so per-feature constants are per-partition scalars):

  z   = h2 @ W1nl               (PE fp8 DoubleRow, psum f32)
  tv  = max(z + (b1-tc), -tc)   (DVE, one pass psum->sbuf fp8)
  yT  = Wbig^T h2T + W2nl^T tv  (PE fp8 DoubleRow, accumulated in psum)
  y   = psum + Cfull -> bf16    (ACT Identity-with-bias / DVE / Pool)
  DMA out [D, rows] bf16; host transposes and upcasts.

Rows are sharded over 8 cores; weights replicated.  ~1.06 MB in +
1 MB out per core.  tc/b1 constants are folded on the host in f64.
"""

import numpy as np
from contextlib import ExitStack

import ml_dtypes
import concourse.bass as bass
import concourse.mybir as mybir
import concourse.tile as tile
from concourse import bacc
from concourse.bass_utils import run_bass_kernel_spmd

N = 8192
D = 512
H1 = 1024
NCORES = 8
RPC = N // NCORES      # rows per core
NLP = 256              # nonlinear hidden columns, padded to 2 chunks
EPS = 1e-5
N_WARMUP = 7
KC = D // 128          # 4 contraction chunks over D
DC = D // 128          # 4 output chunks over D
NLC = NLP // 128       # 2
HALF = 512             # rows per psum group

BF16 = mybir.dt.bfloat16
F32 = mybir.dt.float32
F8 = mybir.dt.float8e4
NPF8 = np.dtype(ml_dtypes.float8_e4m3)
NPBF16 = np.dtype(ml_dtypes.bfloat16)
DR = mybir.MatmulPerfMode.DoubleRow


def build_bass():
    nc = bacc.Bacc(
        "TRN2", target_bir_lowering=False, debug=False, num_devices=NCORES
    )
    HX = nc.dram_tensor("hx", [D, RPC], F8, kind="ExternalInput")
    WB = nc.dram_tensor("wb", [D, D], F8, kind="ExternalInput")
    W1N = nc.dram_tensor("w1n", [D, NLP], F8, kind="ExternalInput")
    W2N = nc.dram_tensor("w2n", [NLP, D], F8, kind="ExternalInput")
    CST = nc.dram_tensor("cst", [128, 2 * NLC + DC], F32, kind="ExternalInput")
    Y = nc.dram_tensor("y", [D, RPC], BF16, kind="ExternalOutput")

    with ExitStack() as ctx:
        tc = ctx.enter_context(tile.TileContext(nc))
        consts = ctx.enter_context(tc.tile_pool(name="consts", bufs=1))
        acts = ctx.enter_context(tc.tile_pool(name="acts", bufs=1))
        zpsum = ctx.enter_context(tc.tile_pool(name="zpsum", bufs=4, space="PSUM"))
        ypsum = ctx.enter_context(tc.tile_pool(name="ypsum", bufs=4, space="PSUM"))
        ypool = ctx.enter_context(tc.tile_pool(name="ypool", bufs=4))

        # PE warm-up on a memset tile (no DMA dependency): fills the HAM
        # activity window during the load phase so real matmuls are not
        # frequency-throttled.
        wa = consts.tile([128, 512], BF16)
        nc.vector.memset(wa[:], 0.0)
        wp = ypsum.tile([128, HALF], F32, tag="yp")
        for _ in range(N_WARMUP):
            nc.tensor.matmul(wp[:], wa[:, :128], wa[:], start=True, stop=True)

        # streaming inputs, critical-path order, one trigger per tensor
        cstsb = consts.tile([128, 2 * NLC + DC], F32)
        nc.sync.dma_start(cstsb[:], CST[:, :])
        w1nsb = consts.tile([128, KC, NLP], F8)
        W1r = W1N.rearrange("(kc p) n -> p kc n", p=128)
        nc.sync.dma_start(w1nsb[:], W1r[:, :, :])
        h2sb = acts.tile([128, KC, RPC], F8)
        H2r = HX.rearrange("(kc p) r -> p kc r", p=128)
        nc.sync.dma_start(h2sb[:, :, 0:HALF], H2r[:, :, 0:HALF])
        wbsb = consts.tile([128, KC, D], F8)
        WBr = WB.rearrange("(kc p) d -> p kc d", p=128)
        nc.sync.dma_start(wbsb[:], WBr[:, :, :])
        w2nsb = consts.tile([128, NLC, D], F8)
        W2r = W2N.rearrange("(kc p) d -> p kc d", p=128)
        nc.sync.dma_start(w2nsb[:], W2r[:, :, :])
        nc.sync.dma_start(h2sb[:, :, HALF:RPC], H2r[:, :, HALF:RPC])

        b1mtc = cstsb[:, 0:NLC]
        ntc = cstsb[:, NLC : 2 * NLC]
        cf = cstsb[:, 2 * NLC : 2 * NLC + DC]

        tvsb = acts.tile([128, NLC, RPC], F8)
        Yr = Y.rearrange("(dc p) r -> dc p r", p=128)

        # z = h2 @ W1nl; tv = max(z + (b1 - tc), -tc) in one DVE pass
        for hf in range(2):
            rs = hf * HALF
            for nl in range(NLC):
                zp = zpsum.tile([128, HALF], F32, tag="zp")
                for p in range(KC // 2):
                    nc.tensor.matmul(
                        zp[:],
                        w1nsb[:, 2 * p : 2 * p + 2, nl * 128 : (nl + 1) * 128],
                        h2sb[:, 2 * p : 2 * p + 2, rs : rs + HALF],
                        start=(p == 0),
                        stop=(p == KC // 2 - 1),
                        perf_mode=DR,
                    )
                nc.vector.tensor_scalar(
                    tvsb[:, nl, rs : rs + HALF],
                    zp[:],
                    b1mtc[:, nl : nl + 1],
                    ntc[:, nl : nl + 1],
                    mybir.AluOpType.add,
                    mybir.AluOpType.max,
                )

        # yT[dc] = sum_kc Wbig^T h2 + W2nl^T tv  (+Cfull, ->bf16)
        ysb = [
            ypool.tile([128, RPC], BF16, tag=f"ysb{dc}", name=f"ysb{dc}")
            for dc in range(DC)
        ]
        for hf in range(2):
            rs = hf * HALF
            for dc in range(DC):
                yp = ypsum.tile([128, HALF], F32, tag="yp")
                for p in range(KC // 2):
                    nc.tensor.matmul(
                        yp[:],
                        wbsb[:, 2 * p : 2 * p + 2, dc * 128 : (dc + 1) * 128],
                        h2sb[:, 2 * p : 2 * p + 2, rs : rs + HALF],
                        start=(p == 0),
                        stop=False,
                        perf_mode=DR,
                    )
                nc.tensor.matmul(
                    yp[:],
                    w2nsb[:, :, dc * 128 : (dc + 1) * 128],
                    tvsb[:, :, rs : rs + HALF],
                    start=False,
                    stop=True,
                    perf_mode=DR,
                )
                # out-stage alternates Scalar/Vector (only they read PSUM)
                if (hf * DC + dc) % 2 == 0:
                    nc.scalar.activation(
                        ysb[dc][:, rs : rs + HALF],
                        yp[:],
                        mybir.ActivationFunctionType.Identity,
                        bias=cf[:, dc : dc + 1],
                        scale=1.0,
                    )
                else:
                    nc.vector.tensor_scalar(
                        ysb[dc][:, rs : rs + HALF], yp[:],
                        cf[:, dc : dc + 1], None, mybir.AluOpType.add,
                    )
                if hf == 1:
                    nc.sync.dma_start(Yr[dc], ysb[dc][:])
    nc.compile()
    return nc


_CACHE = {}


def _get_bass():
    if "nc" not in _CACHE:
        _CACHE["nc"] = build_bass()
    return _CACHE["nc"]


def _host_fold(inputs):
    """Fold attention shortcut + BNs, classify relu columns (f64)."""
    f = lambda k: inputs[k].astype(np.float64)
    h = f("h")
    a1 = f("bn1_g") / np.sqrt(f("bn1_v") + EPS)
    c1 = f("bn1_b") - f("bn1_m") * a1
    a2 = f("bn2_g") / np.sqrt(f("bn2_v") + EPS)
    c2 = f("bn2_b") - f("bn2_m") * a2

    hs = h.sum(axis=0)
    s = hs @ f("vw") + N * f("vb")           # column sums of v
    base = s @ f("ow") + f("ob")             # constant attention-out row
    d1 = base * a1 + c1
    sP = a1 * a2

    W1 = (1.0 / a2)[:, None] * f("f1w")
    b1 = d1 @ f("f1w") + f("f1b")
    W2 = f("f2w") * a2[None, :]
    C0 = (d1 + f("f2b")) * a2 + c2
    h2 = h * sP[None, :]
    tc = np.maximum(b1, 0.0)
    Cfull = C0 + tc @ W2

    # rigorous per-column relu state via Cauchy-Schwarz over actual rows
    maxr = np.sqrt((h2 * h2).sum(axis=1)).max()
    tau = maxr * np.sqrt((W1 * W1).sum(axis=0))
    on = b1 >= tau
    off = b1 <= -tau
    nl_idx = np.where(~(on | off))[0]
    assert len(nl_idx) <= NLP, len(nl_idx)

    Wbig = np.eye(D) + W1[:, on] @ W2[on, :]
    W1n = np.zeros((D, NLP))
    W1n[:, : len(nl_idx)] = W1[:, nl_idx]
    W2n = np.zeros((NLP, D))
    W2n[: len(nl_idx), :] = W2[nl_idx, :]
    b1n = np.zeros(NLP)
    b1n[: len(nl_idx)] = b1[nl_idx]
    tcn = np.zeros(NLP)
    tcn[: len(nl_idx)] = tc[nl_idx]

    pack = lambda v, c: np.ascontiguousarray(
        v.astype(np.float32).reshape(c, 128).T
    )
    cst = np.concatenate(
        [pack(b1n - tcn, NLC), pack(-tcn, NLC), pack(Cfull, DC)], axis=1
    )
    return {
        "h2q": h2.astype(np.float32).astype(NPF8),
        "wb": Wbig.astype(np.float32).astype(NPF8),
        "w1n": W1n.astype(np.float32).astype(NPF8),
        "w2n": W2n.astype(np.float32).astype(NPF8),
        "cst": np.ascontiguousarray(cst),
    }


def make_in_maps(inputs):
    hf = _host_fold(inputs)
    in_maps = []
    for c in range(NCORES):
        r0 = c * RPC
        in_maps.append(
            {
                "hx": np.ascontiguousarray(hf["h2q"][r0 : r0 + RPC].T),
                "wb": hf["wb"],
                "w1n": hf["w1n"],
                "w2n": hf["w2n"],
                "cst": hf["cst"],
            }
        )
    return in_maps


def kernel(**inputs):
    nc = _get_bass()
    in_maps = make_in_maps(inputs)
    res = run_bass_kernel_spmd(nc, in_maps, core_ids=list(range(NCORES)))
    out = np.empty((N, D), np.float32)
    for c in range(NCORES):
        out[c * RPC : (c + 1) * RPC, :] = res.results[c]["y"].T.astype(np.float32)
    return out
